# revision 8
# baseline (speedup 1.0000x reference)
"""2-layer GCN (nn_Net_22101901705332) on 8 Trainium2 NeuronCores.

Strategy (1D node partitioning, edges bucketed by destination):
  - host: add self-loops, compute D^-1/2 A D^-1/2 edge norms, bucket edges by
    dst-owner core, split per core into 4 source blocks of 25000 rows (so
    gather indices fit int16), sort by dst, group into 128-row dst windows,
    pad each (block, window) group to a multiple of 128 edges (counts
    uniformized across cores so one SPMD program serves all 8).
  - device, per core c (owns nodes [12500c, 12500(c+1))):
      A) s_own = x_own @ W1 (bf16)  -> AllGather -> s_full (bf16)
      B) layer-1 aggregation: dma_gather s rows by edge src (4 SWDGE queues),
         segment-sum via TensorE: psum_w += Sel_chunk^T @ gathered_chunk where
         Sel[i, j] = norm_i * one_hot(dst_i)[j]  (host-precomputed, streamed),
         + b1 via ones-row matmul, ReLU on PSUM flush -> h (bf16, DRAM)
      C) t_own = h_own @ W2 via DMA-transposed h; AllGather -> t_full
         (t rows padded to 128 cols so the 256-byte gather constraint holds)
      D) layer-2 aggregation (same indices + same Sel buffer), + b2,
         log_softmax per row -> o_own (f32)
  - host: concatenate the 8 output shards.
"""

import numpy as np
import ml_dtypes

import concourse.bass as bass
import concourse.mybir as mybir
from concourse import bacc, tile
from concourse.bass_utils import run_bass_kernel_spmd

BF16 = ml_dtypes.bfloat16

N = 100000
E = 1600000
F, H, C = 512, 256, 64
P = 8                     # cores
PIECE = 2048              # gather idxs per dma_gather call (16 chunks)
SELP = 16                 # sel chunks per stream piece


def _set_dims():
    global NOWN, NB, BS, W, NW, LASTW, PPC
    NOWN = N // P             # nodes per core
    NB = 4                    # src blocks
    BS = N // NB              # rows per block (int16-safe)
    W = 128                   # dst window rows (psum partitions)
    NW = -(-NOWN // W)        # windows; last may be partial
    LASTW = NOWN - (NW - 1) * W
    PPC = PIECE // 128        # chunks per gather piece


_set_dims()


def _preprocess(x, edge_index, edge_weight, W1, b1, W2, b2):
    x = np.asarray(x, dtype=np.float32)
    ei = np.asarray(edge_index).astype(np.int64)
    ew = np.asarray(edge_weight, dtype=np.float32)

    src = np.concatenate([ei[0], np.arange(N, dtype=np.int64)])
    dst = np.concatenate([ei[1], np.arange(N, dtype=np.int64)])
    wgt = np.concatenate([ew, np.ones(N, np.float32)]).astype(np.float32)
    deg = np.bincount(dst, weights=wgt, minlength=N).astype(np.float32)
    dinv = np.where(deg > 0, 1.0 / np.sqrt(deg), 0.0).astype(np.float32)
    norm = (dinv[src] * wgt * dinv[dst]).astype(np.float32)

    owner = dst // NOWN
    cores = []
    counts = np.zeros((P, NB, NW), np.int64)
    for c in range(P):
        m = owner == c
        es, ed, en = src[m], dst[m] - c * NOWN, norm[m]
        blk = es // BS
        wid = ed // W
        order = np.lexsort((ed, wid, blk))
        es, ed, en, blk, wid = es[order], ed[order], en[order], blk[order], wid[order]
        cnt = np.bincount(blk * NW + wid, minlength=NB * NW).reshape(NB, NW)
        counts[c] = cnt
        cores.append((es, ed, en, blk, wid, cnt))

    # uniform chunk counts across cores (SPMD): K[b, w] 128-edge chunks
    K = -(-counts.max(axis=0) // 128)          # [NB, NW]
    cumKb = np.cumsum(np.concatenate([np.zeros((NB, 1), np.int64), K], axis=1), axis=1)  # excl prefix per block
    totKw = K.sum(axis=0)                      # chunks per window
    GCW = np.concatenate([[0], np.cumsum(totKw)])  # global chunk base per window
    preB = np.cumsum(np.concatenate([np.zeros((1, NW), np.int64), K], axis=0), axis=0)  # [NB+1, NW]
    TOTC = int(K.sum())
    S = [int(K[b].sum()) * 128 for b in range(NB)]

    # bf16 weights, transposed/reshaped for the device
    W1b = np.asarray(W1, np.float32).astype(BF16)            # [512, 256]
    W2b = np.asarray(W2, np.float32).astype(BF16)            # [256, 64]
    w1kc = W1b.reshape(4, 128, H).transpose(1, 0, 2).copy()  # [128, 4, 256]
    w2kc = W2b.reshape(2, 128, C).transpose(1, 0, 2).copy()  # [128, 2, 64]
    b1r = np.asarray(b1, np.float32).astype(BF16).reshape(1, H)
    b2r = np.asarray(b2, np.float32).astype(BF16).reshape(1, C)
    xT = np.ascontiguousarray(x.T).astype(BF16)              # [512, 100000]

    in_maps = []
    for c in range(P):
        es, ed, en, blk, wid, cnt = cores[c]
        im = {"w1kc": w1kc, "w2kc": w2kc, "b1r": b1r, "b2r": b2r,
              "xT_own": np.ascontiguousarray(xT[:, c * NOWN:(c + 1) * NOWN])}
        # position of each real edge inside its (b, w) group
        key = blk * NW + wid
        grp_start = np.concatenate([[0], np.cumsum(cnt.reshape(-1))])[:-1]
        pos_in_grp = np.arange(len(es)) - grp_start[key]
        # destination slot inside the per-block padded stream
        dest = 128 * cumKb[blk, wid] + pos_in_grp
        sel = np.zeros((128, TOTC, 128), BF16)
        loc_chunk = dest // 128
        g_slot = GCW[wid] + preB[blk, wid] + (loc_chunk - cumKb[blk, wid])
        sel[dest % 128, g_slot, ed - wid * W] = en.astype(BF16)
        im["sel"] = sel
        for b in range(NB):
            idx = np.zeros(S[b], np.int16)
            mb = blk == b
            idx[dest[mb]] = (es[mb] - b * BS).astype(np.int16)
            wrapped = idx.reshape(S[b] // 16, 16).T            # [16, S/16]
            im[f"idx{b}"] = np.ascontiguousarray(np.tile(wrapped, (8, 1)))
        in_maps.append(im)

    sched = dict(K=K, cumKb=cumKb, GCW=GCW, preB=preB, TOTC=TOTC, S=S)
    return in_maps, sched


def _emit_agg(nc, tc, pools, sched, layer, src_dram, b_tile, ones_t, idx_tiles,
              sel_dram, h_dram, o_own):
    """Emit one aggregation layer: gather + Sel matmuls + per-window flush."""
    K, cumKb = sched["K"], sched["cumKb"]
    S, TOTC = sched["S"], sched["TOTC"]
    gpool, selpool, fpool, psum = pools
    pool = fpool
    ELEMS = H if layer == 1 else 128          # gathered row length (elems)
    RHSW = H if layer == 1 else C             # matmul N dim
    tag = "L%d" % layer

    cur_piece = [-1] * NB
    gtiles = [None] * NB
    sel_cur = [-1]
    sel_tile = [None]

    def gather_piece(b, piece):
        n = min(PIECE, S[b] - piece * PIECE)
        t = gpool.tile([128, n // 128, ELEMS], mybir.dt.bfloat16, tag=f"g{b}")
        nc.gpsimd.dma_gather(
            t[:], src_dram[b * BS:(b + 1) * BS, :],
            idx_tiles[b][:, (piece * PIECE) // 16:(piece * PIECE + n) // 16],
            n, n, ELEMS, queue_num=b, single_packet=False,
        )
        return t

    def sel_piece(piece):
        n = min(SELP, TOTC - piece * SELP)
        t = selpool.tile([128, n, 128], mybir.dt.bfloat16, tag="sel")
        nc.sync.dma_start(t[:], sel_dram[:, piece * SELP:piece * SELP + n, :])
        return t

    g = 0
    for w in range(NW):
        acc = psum.tile([128, RHSW], mybir.dt.float32, tag="acc")
        i = 0
        for b in range(NB):
            for k in range(int(K[b, w])):
                bj = int(cumKb[b, w]) + k
                piece, sub = bj // PPC, bj % PPC
                if piece != cur_piece[b]:
                    gtiles[b] = gather_piece(b, piece)
                    cur_piece[b] = piece
                sp, ss = g // SELP, g % SELP
                if sp != sel_cur[0]:
                    sel_tile[0] = sel_piece(sp)
                    sel_cur[0] = sp
                nc.tensor.matmul(acc[:], sel_tile[0][:, ss, :],
                                 gtiles[b][:, sub, 0:RHSW],
                                 start=(i == 0), stop=False)
                i += 1
                g += 1
        # bias via ones-row matmul (K=1)
        nc.tensor.matmul(acc[:], ones_t[0:1, :], b_tile[0:1, :],
                         start=(i == 0), stop=True)

        rows = W if w < NW - 1 else LASTW
        if layer == 1:
            ht = pool.tile([128, H], mybir.dt.bfloat16, tag="hflush")
            nc.scalar.activation(ht[:], acc[:], mybir.ActivationFunctionType.Relu)
            nc.sync.dma_start(h_dram[w * W:(w + 1) * W, :], ht[:])
        else:
            # log_softmax over the 64 classes, straight off PSUM
            mx = pool.tile([128, 1], mybir.dt.float32, tag="lsm")
            nc.vector.tensor_reduce(mx[0:rows], acc[0:rows], mybir.AxisListType.X,
                                    mybir.AluOpType.max)
            nmx = pool.tile([128, 1], mybir.dt.float32, tag="lsn")
            nc.vector.tensor_scalar_mul(nmx[0:rows], mx[0:rows], -1.0)
            et = pool.tile([128, C], mybir.dt.float32, tag="lse")
            se = pool.tile([128, 1], mybir.dt.float32, tag="lss")
            nc.scalar.activation(et[0:rows], acc[0:rows],
                                 mybir.ActivationFunctionType.Exp,
                                 bias=nmx[0:rows], accum_out=se[0:rows])
            ln = pool.tile([128, 1], mybir.dt.float32, tag="lsl")
            nc.scalar.activation(ln[0:rows], se[0:rows],
                                 mybir.ActivationFunctionType.Ln)
            cc = pool.tile([128, 1], mybir.dt.float32, tag="lsc")
            nc.vector.tensor_sub(cc[0:rows], nmx[0:rows], ln[0:rows])
            ot = pool.tile([128, C], mybir.dt.float32, tag="lso")
            nc.scalar.activation(ot[0:rows], acc[0:rows],
                                 mybir.ActivationFunctionType.Identity,
                                 bias=cc[0:rows])
            nc.sync.dma_start(o_own[w * W:w * W + rows, :], ot[0:rows, :])


def _build_program(sched):
    K, S, TOTC = sched["K"], sched["S"], sched["TOTC"]
    nc = bacc.Bacc(None, target_bir_lowering=False, debug=False,
                   num_devices=P, num_swdge_queues=4)

    xT_own = nc.dram_tensor("xT_own", [F, NOWN], mybir.dt.bfloat16, kind="ExternalInput")
    w1kc = nc.dram_tensor("w1kc", [128, 4, H], mybir.dt.bfloat16, kind="ExternalInput")
    w2kc = nc.dram_tensor("w2kc", [128, 2, C], mybir.dt.bfloat16, kind="ExternalInput")
    b1r = nc.dram_tensor("b1r", [1, H], mybir.dt.bfloat16, kind="ExternalInput")
    b2r = nc.dram_tensor("b2r", [1, C], mybir.dt.bfloat16, kind="ExternalInput")
    sel_dram = nc.dram_tensor("sel", [128, TOTC, 128], mybir.dt.bfloat16, kind="ExternalInput")
    idx_dram = [nc.dram_tensor(f"idx{b}", [128, S[b] // 16], mybir.dt.int16, kind="ExternalInput")
                for b in range(NB)]
    o_own = nc.dram_tensor("o_own", [NOWN, C], mybir.dt.float32, kind="ExternalOutput")

    s_own = nc.dram_tensor("s_own", [NOWN, H], mybir.dt.bfloat16)
    s_full = nc.dram_tensor("s_full", [N, H], mybir.dt.bfloat16, addr_space="Shared")
    HPAD = NW * W                                            # 12544
    h_dram = nc.dram_tensor("h_dram", [HPAD, H], mybir.dt.bfloat16)
    t_own = nc.dram_tensor("t_own", [NOWN, 128], mybir.dt.bfloat16)
    t_full = nc.dram_tensor("t_full", [N, 128], mybir.dt.bfloat16, addr_space="Shared")

    with tile.TileContext(nc) as tc:
        with (
            tc.tile_pool(name="const", bufs=1) as constp,
            tc.tile_pool(name="idxp", bufs=1) as idxp,
            tc.tile_pool(name="pool", bufs=2) as pool,
            tc.tile_pool(name="gpool", bufs=2) as gpool,
            tc.tile_pool(name="selpool", bufs=3) as selpool,
            tc.tile_pool(name="fpool", bufs=3) as fpool,
            tc.tile_pool(name="hT", bufs=1) as hTp,
            tc.tile_pool(name="psum", bufs=4, space="PSUM") as psum,
        ):
            w1_t = constp.tile([128, 4, H], mybir.dt.bfloat16)
            nc.sync.dma_start(w1_t[:], w1kc[:])
            w2_t = constp.tile([128, 2, C], mybir.dt.bfloat16)
            nc.sync.dma_start(w2_t[:], w2kc[:])
            b1_t = constp.tile([1, H], mybir.dt.bfloat16)
            nc.sync.dma_start(b1_t[:], b1r[:])
            b2_t = constp.tile([1, C], mybir.dt.bfloat16)
            nc.sync.dma_start(b2_t[:], b2r[:])
            ones_t = constp.tile([1, 128], mybir.dt.bfloat16)
            nc.vector.memset(ones_t[:], 1.0)
            idx_tiles = []
            for b in range(NB):
                it = idxp.tile([128, S[b] // 16], mybir.dt.int16, tag=f"idx{b}")
                nc.sync.dma_start(it[:], idx_dram[b][:])
                idx_tiles.append(it)

            # ---- phase A: s_own = x_own @ W1 ----
            nt = 0
            while nt * W < NOWN:
                span = min(4, NW - nt)          # node sub-tiles in this big tile
                cols = min(4 * W, NOWN - nt * W)
                xt = pool.tile([128, 4, cols], mybir.dt.bfloat16, tag="xt")
                nc.sync.dma_start(
                    xt[:, :, 0:cols],
                    xT_own[:, nt * W:nt * W + cols].rearrange("(c p) n -> p c n", p=128))
                for s_ in range(span):
                    rows = min(W, cols - s_ * W)
                    acc = psum.tile([128, H], mybir.dt.float32, tag="acc")
                    for kc in range(4):
                        nc.tensor.matmul(acc[0:rows, :],
                                         xt[:, kc, s_ * W:s_ * W + rows],
                                         w1_t[:, kc, :],
                                         start=(kc == 0), stop=(kc == 3))
                    st = pool.tile([128, H], mybir.dt.bfloat16, tag="sflush")
                    nc.scalar.activation(st[0:rows], acc[0:rows],
                                         mybir.ActivationFunctionType.Copy)
                    nc.sync.dma_start(s_own[(nt + s_) * W:(nt + s_) * W + rows, :],
                                      st[0:rows, :])
                nt += span

            nc.gpsimd.collective_compute(
                "AllGather", mybir.AluOpType.bypass,
                replica_groups=[list(range(P))],
                ins=[s_own[:]], outs=[s_full[:]])

            # ---- phase B: layer-1 aggregation -> h ----
            _emit_agg(nc, tc, (gpool, selpool, fpool, psum), sched, 1, s_full,
                      b1_t, ones_t, idx_tiles, sel_dram, h_dram, o_own)

            # ---- phase C: t_own = relu(h) @ W2, AllGather ----
            hT0 = hTp.tile([128, HPAD], mybir.dt.bfloat16)
            nc.sync.dma_start(hT0[:], h_dram[:, 0:128], transpose=True)
            hT1 = hTp.tile([128, HPAD], mybir.dt.bfloat16)
            nc.sync.dma_start(hT1[:], h_dram[:, 128:256], transpose=True)
            for i in range(NW):
                rows = W if i < NW - 1 else LASTW
                acc = psum.tile([128, C], mybir.dt.float32, tag="acc")
                nc.tensor.matmul(acc[0:rows, :], hT0[:, i * W:i * W + rows],
                                 w2_t[:, 0, :], start=True, stop=False)
                nc.tensor.matmul(acc[0:rows, :], hT1[:, i * W:i * W + rows],
                                 w2_t[:, 1, :], start=False, stop=True)
                tt = pool.tile([128, 128], mybir.dt.bfloat16, tag="tflush")
                nc.vector.memset(tt[:, C:128], 0.0)
                nc.scalar.activation(tt[0:rows, 0:C], acc[0:rows, :],
                                     mybir.ActivationFunctionType.Copy)
                nc.sync.dma_start(t_own[i * W:i * W + rows, :], tt[0:rows, :])

            nc.gpsimd.collective_compute(
                "AllGather", mybir.AluOpType.bypass,
                replica_groups=[list(range(P))],
                ins=[t_own[:]], outs=[t_full[:]])

            # ---- phase D: layer-2 aggregation + log_softmax -> o_own ----
            _emit_agg(nc, tc, (gpool, selpool, fpool, psum), sched, 2, t_full,
                      b2_t, ones_t, idx_tiles, sel_dram, h_dram, o_own)

    nc.compile()
    return nc


def _run(inputs, trace=False, trace_kwargs=None):
    in_maps, sched = _preprocess(**inputs)
    nc = _build_program(sched)
    r = run_bass_kernel_spmd(nc, in_maps, core_ids=list(range(P)),
                             trace=trace, **(trace_kwargs or {}))
    out = np.concatenate([r.results[c]["o_own"] for c in range(P)], axis=0)
    return out, r


def kernel(**inputs) -> np.ndarray:
    out, _ = _run(inputs)
    return out


# revision 13
# speedup vs baseline: 1.1618x; 1.1618x over previous
"""2-layer GCN (nn_Net_22101901705332) on 8 Trainium2 NeuronCores.

Strategy (1D node partitioning, edges bucketed by destination):
  - host: add self-loops, compute D^-1/2 A D^-1/2 edge norms, bucket edges by
    dst-owner core, split per core into 4 source blocks of 25000 rows (so
    gather indices fit int16), sort by dst, group into 128-row dst windows,
    pad each (block, window) group to a multiple of 128 edges (counts
    uniformized across cores so one SPMD program serves all 8).
  - device, per core c (owns nodes [12500c, 12500(c+1))):
      A) s_own = x_own @ W1 (bf16)  -> AllGather -> s_full (bf16)
      B) layer-1 aggregation: dma_gather s rows by edge src (4 SWDGE queues),
         segment-sum via TensorE: psum_w += Sel_chunk^T @ gathered_chunk where
         Sel[i, j] = norm_i * one_hot(dst_i)[j]  (host-precomputed, streamed),
         + b1 via ones-row matmul, ReLU on PSUM flush -> h (bf16, DRAM)
      C) t_own = h_own @ W2 via DMA-transposed h; AllGather -> t_full
         (t rows padded to 128 cols so the 256-byte gather constraint holds)
      D) layer-2 aggregation (same indices + same Sel buffer), + b2,
         log_softmax per row -> o_own (f32)
  - host: concatenate the 8 output shards.
"""

import numpy as np
import ml_dtypes

import concourse.bass as bass
import concourse.mybir as mybir
from concourse import bacc, tile
from concourse.bass_utils import run_bass_kernel_spmd

BF16 = ml_dtypes.bfloat16

N = 100000
E = 1600000
F, H, C = 512, 256, 64
P = 8                     # cores
PIECE = 2048              # gather idxs per dma_gather call (16 chunks)
SELP = 16                 # sel chunks per stream piece


def _set_dims():
    global NOWN, NB, BS, W, NW, LASTW, PPC
    NOWN = N // P             # nodes per core
    NB = 4                    # src blocks
    BS = N // NB              # rows per block (int16-safe)
    W = 128                   # dst window rows (psum partitions)
    NW = -(-NOWN // W)        # windows; last may be partial
    LASTW = NOWN - (NW - 1) * W
    PPC = PIECE // 128        # chunks per gather piece


_set_dims()


def _preprocess(x, edge_index, edge_weight, W1, b1, W2, b2):
    x = np.asarray(x, dtype=np.float32)
    ei = np.asarray(edge_index).astype(np.int64)
    ew = np.asarray(edge_weight, dtype=np.float32)

    src = np.concatenate([ei[0], np.arange(N, dtype=np.int64)])
    dst = np.concatenate([ei[1], np.arange(N, dtype=np.int64)])
    wgt = np.concatenate([ew, np.ones(N, np.float32)]).astype(np.float32)
    deg = np.bincount(dst, weights=wgt, minlength=N).astype(np.float32)
    dinv = np.where(deg > 0, 1.0 / np.sqrt(deg), 0.0).astype(np.float32)
    norm = (dinv[src] * wgt * dinv[dst]).astype(np.float32)

    owner = dst // NOWN
    cores = []
    counts = np.zeros((P, NB, NW), np.int64)
    for c in range(P):
        m = owner == c
        es, ed, en = src[m], dst[m] - c * NOWN, norm[m]
        blk = es // BS
        wid = ed // W
        order = np.lexsort((ed, wid, blk))
        es, ed, en, blk, wid = es[order], ed[order], en[order], blk[order], wid[order]
        cnt = np.bincount(blk * NW + wid, minlength=NB * NW).reshape(NB, NW)
        counts[c] = cnt
        cores.append((es, ed, en, blk, wid, cnt))

    # uniform chunk counts across cores (SPMD): K[b, w] 128-edge chunks
    K = -(-counts.max(axis=0) // 128)          # [NB, NW]
    cumKb = np.cumsum(np.concatenate([np.zeros((NB, 1), np.int64), K], axis=1), axis=1)  # excl prefix per block
    totKw = K.sum(axis=0)                      # chunks per window
    GCW = np.concatenate([[0], np.cumsum(totKw)])  # global chunk base per window
    preB = np.cumsum(np.concatenate([np.zeros((1, NW), np.int64), K], axis=0), axis=0)  # [NB+1, NW]
    TOTC = int(K.sum())
    S = [int(K[b].sum()) * 128 for b in range(NB)]

    # bf16 weights, transposed/reshaped for the device
    W1b = np.asarray(W1, np.float32).astype(BF16)            # [512, 256]
    W2b = np.asarray(W2, np.float32).astype(BF16)            # [256, 64]
    w1kc = W1b.reshape(4, 128, H).transpose(1, 0, 2).copy()  # [128, 4, 256]
    w2kc = W2b.reshape(2, 128, C).transpose(1, 0, 2).copy()  # [128, 2, 64]
    b1r = np.asarray(b1, np.float32).astype(BF16).reshape(1, H)
    b2r = np.asarray(b2, np.float32).astype(BF16).reshape(1, C)
    xT = np.ascontiguousarray(x.T).astype(BF16)              # [512, 100000]

    in_maps = []
    for c in range(P):
        es, ed, en, blk, wid, cnt = cores[c]
        im = {"w1kc": w1kc, "w2kc": w2kc, "b1r": b1r, "b2r": b2r,
              "xT_own": np.ascontiguousarray(xT[:, c * NOWN:(c + 1) * NOWN])}
        # position of each real edge inside its (b, w) group
        key = blk * NW + wid
        grp_start = np.concatenate([[0], np.cumsum(cnt.reshape(-1))])[:-1]
        pos_in_grp = np.arange(len(es)) - grp_start[key]
        # destination slot inside the per-block padded stream
        dest = 128 * cumKb[blk, wid] + pos_in_grp
        loc_chunk = dest // 128
        g_slot = GCW[wid] + preB[blk, wid] + (loc_chunk - cumKb[blk, wid])
        dstrel = np.zeros((128, TOTC), BF16)
        nrm = np.zeros((128, TOTC), BF16)
        dstrel[dest % 128, g_slot] = (ed - wid * W).astype(BF16)
        nrm[dest % 128, g_slot] = en.astype(BF16)
        im["dstrel"] = dstrel
        im["nrm"] = nrm
        im["iota128"] = np.broadcast_to(np.arange(128, dtype=np.float32).astype(BF16),
                                        (128, 128)).copy()
        for b in range(NB):
            idx = np.zeros(S[b], np.int16)
            mb = blk == b
            idx[dest[mb]] = (es[mb] - b * BS).astype(np.int16)
            wrapped = idx.reshape(S[b] // 16, 16).T            # [16, S/16]
            im[f"idx{b}"] = np.ascontiguousarray(np.tile(wrapped, (8, 1)))
        in_maps.append(im)

    sched = dict(K=K, cumKb=cumKb, GCW=GCW, preB=preB, TOTC=TOTC, S=S)
    return in_maps, sched


def _bcast(ap, n, where):
    """Insert a broadcast (step-0) dim of extent n into a 2D AP at position `where` (1 or 2)."""
    a = ap.ap
    if where == 1:
        new = [a[0], [0, n], a[1]]
    else:
        new = [a[0], a[1], [0, n]]
    return bass.AP(ap.tensor, ap.offset, new)


def _emit_agg(nc, tc, pools, sched, layer, src_dram, b_tile, ones_t, idx_tiles,
              meta, h_dram, o_own, oscr=None):
    """Emit one aggregation layer: gather + Sel matmuls + per-window flush."""
    K, cumKb = sched["K"], sched["cumKb"]
    S, TOTC = sched["S"], sched["TOTC"]
    gpool, selpool, fpool, psum = pools
    pool = fpool
    iota_t, dstrel_t, nrm_t = meta
    ELEMS = H if layer == 1 else 128          # gathered row length (elems)
    RHSW = H if layer == 1 else C             # matmul N dim

    cur_piece = [-1] * NB
    gtiles = [None] * NB
    sel_cur = [-1]
    sel_tile = [None]

    def gather_piece(b, piece):
        n = min(PIECE, S[b] - piece * PIECE)
        t = gpool.tile([128, n // 128, ELEMS], mybir.dt.bfloat16, tag=f"g{b}")
        nc.gpsimd.dma_gather(
            t[:], src_dram[b * BS:(b + 1) * BS, :],
            idx_tiles[b][:, (piece * PIECE) // 16:(piece * PIECE + n) // 16],
            n, n, ELEMS, queue_num=b, single_packet=False,
        )
        return t

    def sel_piece(piece):
        # build Sel[k, chunk, m] = nrm * (iota[m] == dstrel) on the vector engine
        n = min(SELP, TOTC - piece * SELP)
        t = selpool.tile([128, n, 128], mybir.dt.bfloat16, tag="sel")
        iota_b = _bcast(iota_t[:], n, 1)
        dr = dstrel_t[:, piece * SELP:piece * SELP + n]
        nr = nrm_t[:, piece * SELP:piece * SELP + n]
        nc.vector.tensor_tensor(t[:], iota_b, _bcast(dr, 128, 2),
                                mybir.AluOpType.is_equal)
        nc.vector.tensor_tensor(t[:], t[:], _bcast(nr, 128, 2),
                                mybir.AluOpType.mult)
        return t

    g = 0
    for w in range(NW):
        acc = psum.tile([128, RHSW], mybir.dt.float32, tag="acc")
        i = 0
        for b in range(NB):
            for k in range(int(K[b, w])):
                bj = int(cumKb[b, w]) + k
                piece, sub = bj // PPC, bj % PPC
                if piece != cur_piece[b]:
                    gtiles[b] = gather_piece(b, piece)
                    cur_piece[b] = piece
                sp, ss = g // SELP, g % SELP
                if sp != sel_cur[0]:
                    sel_tile[0] = sel_piece(sp)
                    sel_cur[0] = sp
                nc.tensor.matmul(acc[:], sel_tile[0][:, ss, :],
                                 gtiles[b][:, sub, 0:RHSW],
                                 start=(i == 0), stop=False)
                i += 1
                g += 1
        # bias via ones-row matmul (K=1)
        nc.tensor.matmul(acc[:], ones_t[0:1, :], b_tile[0:1, :],
                         start=(i == 0), stop=True)

        rows = W if w < NW - 1 else LASTW
        if layer == 1:
            ht = pool.tile([128, H], mybir.dt.bfloat16, tag="hflush")
            nc.vector.tensor_scalar_max(ht[:], acc[:], 0.0)
            nc.sync.dma_start(h_dram[w * W:(w + 1) * W, :], ht[:])
        else:
            # log_softmax: store (o - max) and sum(exp(o - max)); Ln deferred
            o_scr, se_scr = oscr
            mx = pool.tile([128, 1], mybir.dt.float32, tag="lsm")
            nc.vector.tensor_reduce(mx[:], acc[:], mybir.AxisListType.X,
                                    mybir.AluOpType.max)
            nmx = pool.tile([128, 1], mybir.dt.float32, tag="lsn")
            nc.vector.tensor_scalar_mul(nmx[:], mx[:], -1.0)
            et = pool.tile([128, C], mybir.dt.float32, tag="lse")
            nc.scalar.activation(et[:], acc[:],
                                 mybir.ActivationFunctionType.Exp,
                                 bias=nmx[:], accum_out=se_scr[:, w:w + 1])
            nc.vector.tensor_scalar(o_scr[:, w, :], acc[:], mx[:], None,
                                    mybir.AluOpType.subtract)


def _build_program(sched):
    K, S, TOTC = sched["K"], sched["S"], sched["TOTC"]
    nc = bacc.Bacc(None, target_bir_lowering=False, debug=False,
                   num_devices=P, num_swdge_queues=4,
                   dynamic_dma_scratch_size=32768)

    xT_own = nc.dram_tensor("xT_own", [F, NOWN], mybir.dt.bfloat16, kind="ExternalInput")
    w1kc = nc.dram_tensor("w1kc", [128, 4, H], mybir.dt.bfloat16, kind="ExternalInput")
    w2kc = nc.dram_tensor("w2kc", [128, 2, C], mybir.dt.bfloat16, kind="ExternalInput")
    b1r = nc.dram_tensor("b1r", [1, H], mybir.dt.bfloat16, kind="ExternalInput")
    b2r = nc.dram_tensor("b2r", [1, C], mybir.dt.bfloat16, kind="ExternalInput")
    dstrel_d = nc.dram_tensor("dstrel", [128, TOTC], mybir.dt.bfloat16, kind="ExternalInput")
    nrm_d = nc.dram_tensor("nrm", [128, TOTC], mybir.dt.bfloat16, kind="ExternalInput")
    iota_d = nc.dram_tensor("iota128", [128, 128], mybir.dt.bfloat16, kind="ExternalInput")
    idx_dram = [nc.dram_tensor(f"idx{b}", [128, S[b] // 16], mybir.dt.int16, kind="ExternalInput")
                for b in range(NB)]
    o_own = nc.dram_tensor("o_own", [NOWN, C], mybir.dt.float32, kind="ExternalOutput")

    s_own = nc.dram_tensor("s_own", [NOWN, H], mybir.dt.bfloat16)
    s_full = nc.dram_tensor("s_full", [N, H], mybir.dt.bfloat16, addr_space="Shared")
    HPAD = NW * W                                            # 12544
    h_dram = nc.dram_tensor("h_dram", [HPAD, H], mybir.dt.bfloat16)
    t_own = nc.dram_tensor("t_own", [NOWN, 128], mybir.dt.bfloat16)
    t_full = nc.dram_tensor("t_full", [N, 128], mybir.dt.bfloat16, addr_space="Shared")

    with tile.TileContext(nc) as tc:
        with (
            tc.tile_pool(name="const", bufs=1) as constp,
            tc.tile_pool(name="idxp", bufs=1) as idxp,
            tc.tile_pool(name="pool", bufs=2) as pool,
            tc.tile_pool(name="gpool", bufs=2) as gpool,
            tc.tile_pool(name="selpool", bufs=3) as selpool,
            tc.tile_pool(name="fpool", bufs=3) as fpool,
            tc.tile_pool(name="psum", bufs=4, space="PSUM") as psum,
        ):
            w1_t = constp.tile([128, 4, H], mybir.dt.bfloat16)
            nc.sync.dma_start(w1_t[:], w1kc[:])
            w2_t = constp.tile([128, 2, C], mybir.dt.bfloat16)
            nc.sync.dma_start(w2_t[:], w2kc[:])
            b1_t = constp.tile([1, H], mybir.dt.bfloat16)
            nc.sync.dma_start(b1_t[:], b1r[:])
            b2_t = constp.tile([1, C], mybir.dt.bfloat16)
            nc.sync.dma_start(b2_t[:], b2r[:])
            ones_t = constp.tile([1, 128], mybir.dt.bfloat16)
            nc.vector.memset(ones_t[:], 1.0)
            iota_t = constp.tile([128, 128], mybir.dt.bfloat16)
            nc.sync.dma_start(iota_t[:], iota_d[:])
            dstrel_t = constp.tile([128, TOTC], mybir.dt.bfloat16)
            nc.sync.dma_start(dstrel_t[:], dstrel_d[:])
            nrm_t = constp.tile([128, TOTC], mybir.dt.bfloat16)
            nc.sync.dma_start(nrm_t[:], nrm_d[:])
            meta = (iota_t, dstrel_t, nrm_t)
            idx_tiles = []
            for b in range(NB):
                it = idxp.tile([128, S[b] // 16], mybir.dt.int16, tag=f"idx{b}")
                nc.sync.dma_start(it[:], idx_dram[b][:])
                idx_tiles.append(it)

            # ---- phase A: s_own = x_own @ W1 ----
            nt = 0
            while nt * W < NOWN:
                span = min(4, NW - nt)          # node sub-tiles in this big tile
                cols = min(4 * W, NOWN - nt * W)
                xt = pool.tile([128, 4, cols], mybir.dt.bfloat16, tag="xt")
                nc.sync.dma_start(
                    xt[:, :, 0:cols],
                    xT_own[:, nt * W:nt * W + cols].rearrange("(c p) n -> p c n", p=128))
                for s_ in range(span):
                    rows = min(W, cols - s_ * W)
                    acc = psum.tile([128, H], mybir.dt.float32, tag="acc")
                    for kc in range(4):
                        nc.tensor.matmul(acc[0:rows, :],
                                         xt[:, kc, s_ * W:s_ * W + rows],
                                         w1_t[:, kc, :],
                                         start=(kc == 0), stop=(kc == 3))
                    st = pool.tile([128, H], mybir.dt.bfloat16, tag="sflush")
                    nc.vector.tensor_copy(st[0:rows], acc[0:rows])
                    nc.sync.dma_start(s_own[(nt + s_) * W:(nt + s_) * W + rows, :],
                                      st[0:rows, :])
                nt += span

            nc.gpsimd.collective_compute(
                "AllGather", mybir.AluOpType.bypass,
                replica_groups=[list(range(P))],
                ins=[s_own[:]], outs=[s_full[:]])

            # ---- phase B: layer-1 aggregation -> h ----
            _emit_agg(nc, tc, (gpool, selpool, fpool, psum), sched, 1, s_full,
                      b1_t, ones_t, idx_tiles, meta, h_dram, o_own)

            # ---- phase C: t_own = relu(h) @ W2, AllGather ----
            with tc.tile_pool(name="hT", bufs=1) as hTp:
                hT0 = hTp.tile([128, HPAD], mybir.dt.bfloat16, tag="hT0")
                nc.sync.dma_start(hT0[:], h_dram[:, 0:128], transpose=True)
                hT1 = hTp.tile([128, HPAD], mybir.dt.bfloat16, tag="hT1")
                nc.sync.dma_start(hT1[:], h_dram[:, 128:256], transpose=True)
                for i in range(NW):
                    rows = W if i < NW - 1 else LASTW
                    acc = psum.tile([128, C], mybir.dt.float32, tag="acc")
                    nc.tensor.matmul(acc[0:rows, :], hT0[:, i * W:i * W + rows],
                                     w2_t[:, 0, :], start=True, stop=False)
                    nc.tensor.matmul(acc[0:rows, :], hT1[:, i * W:i * W + rows],
                                     w2_t[:, 1, :], start=False, stop=True)
                    tt = pool.tile([128, 128], mybir.dt.bfloat16, tag="tflush")
                    nc.vector.memset(tt[:, C:128], 0.0)
                    nc.vector.tensor_copy(tt[0:rows, 0:C], acc[0:rows, :])
                    nc.sync.dma_start(t_own[i * W:i * W + rows, :], tt[0:rows, :])

            nc.gpsimd.collective_compute(
                "AllGather", mybir.AluOpType.bypass,
                replica_groups=[list(range(P))],
                ins=[t_own[:]], outs=[t_full[:]])

            # ---- phase D: layer-2 aggregation + log_softmax -> o_own ----
            with tc.tile_pool(name="oscr", bufs=1) as oscrp:
                o_scr = oscrp.tile([128, NW, C], mybir.dt.float32, tag="oscr")
                se_scr = oscrp.tile([128, NW], mybir.dt.float32, tag="sescr")
                _emit_agg(nc, tc, (gpool, selpool, fpool, psum), sched, 2, t_full,
                          b2_t, ones_t, idx_tiles, meta, h_dram, o_own,
                          oscr=(o_scr, se_scr))
                # deferred log-sum-exp: one Ln over all windows, one subtract
                ln_t = oscrp.tile([128, NW], mybir.dt.float32, tag="lnall")
                nc.scalar.activation(ln_t[:], se_scr[:],
                                     mybir.ActivationFunctionType.Ln)
                res = oscrp.tile([128, NW, C], mybir.dt.float32, tag="res")
                nc.vector.scalar_tensor_tensor(
                    res[:], o_scr[:], 0.0, _bcast(ln_t[:], C, 2),
                    mybir.AluOpType.add, mybir.AluOpType.subtract)
                nc.sync.dma_start(
                    o_own[0:(NW - 1) * W, :].rearrange("(w p) c -> p w c", p=128),
                    res[:, 0:NW - 1, :])
                nc.sync.dma_start(o_own[(NW - 1) * W:NOWN, :],
                                  res[0:LASTW, NW - 1, :])

    nc.compile()
    return nc


def _run(inputs, trace=False, trace_kwargs=None):
    in_maps, sched = _preprocess(**inputs)
    nc = _build_program(sched)
    r = run_bass_kernel_spmd(nc, in_maps, core_ids=list(range(P)),
                             trace=trace, **(trace_kwargs or {}))
    out = np.concatenate([r.results[c]["o_own"] for c in range(P)], axis=0)
    return out, r


def kernel(**inputs) -> np.ndarray:
    out, _ = _run(inputs)
    return out


# revision 20
# speedup vs baseline: 1.3887x; 1.1953x over previous
"""2-layer GCN (nn_Net_22101901705332) on 8 Trainium2 NeuronCores.

Strategy (1D node partitioning, edges bucketed by destination):
  - host: add self-loops, compute D^-1/2 A D^-1/2 edge norms, bucket edges by
    dst-owner core, split per core into 4 source blocks of 25000 rows (so
    gather indices fit int16), sort by dst, group into 128-row dst windows,
    pad each (block, window) group to a multiple of 128 edges (counts
    uniformized across cores so one SPMD program serves all 8).
  - device, per core c (owns nodes [12500c, 12500(c+1))):
      A) s_own = x_own @ W1 (bf16)  -> AllGather -> s_full (bf16)
      B) layer-1 aggregation: dma_gather s rows by edge src (4 SWDGE queues),
         segment-sum via TensorE: psum_w += Sel_chunk^T @ gathered_chunk where
         Sel[i, j] = norm_i * one_hot(dst_i)[j]  (host-precomputed, streamed),
         + b1 via ones-row matmul, ReLU on PSUM flush -> h (bf16, DRAM)
      C) t_own = h_own @ W2 via DMA-transposed h; AllGather -> t_full
         (t rows padded to 128 cols so the 256-byte gather constraint holds)
      D) layer-2 aggregation (same indices + same Sel buffer), + b2,
         log_softmax per row -> o_own (f32)
  - host: concatenate the 8 output shards.
"""

import numpy as np
import ml_dtypes

import concourse.bass as bass
import concourse.mybir as mybir
from concourse import bacc, tile
from concourse.bass_utils import run_bass_kernel_spmd

BF16 = ml_dtypes.bfloat16

N = 100000
E = 1600000
F, H, C = 512, 256, 64
P = 8                     # cores
PIECE = 2048              # gather idxs per dma_gather call (16 chunks)
SELP = 16                 # sel chunks per stream piece


def _set_dims():
    global NOWN, NB, BS, W, NW, LASTW, PPC, PQ
    NOWN = N // P             # nodes per core
    NB = 4                    # src blocks (= AllGather pieces)
    BS = N // NB              # rows per block (int16-safe)
    PQ = NOWN // NB           # per-core rows contributed to one AG piece
    W = 128                   # dst window rows (psum partitions)
    NW = -(-NOWN // W)        # windows; last may be partial
    LASTW = NOWN - (NW - 1) * W
    PPC = PIECE // 128        # chunks per gather piece


_set_dims()


def _piece_row(src):
    """Map a global node id to (piece, row-within-piece) for the piece-AllGather
    layout: piece p holds rows [owner*PQ + (local % PQ)] for local//PQ == p."""
    owner = src // NOWN
    local = src % NOWN
    return local // PQ, owner * PQ + local % PQ


def _preprocess(x, edge_index, edge_weight, W1, b1, W2, b2):
    x = np.asarray(x, dtype=np.float32)
    ei = np.asarray(edge_index).astype(np.int64)
    ew = np.asarray(edge_weight, dtype=np.float32)

    src = np.concatenate([ei[0], np.arange(N, dtype=np.int64)])
    dst = np.concatenate([ei[1], np.arange(N, dtype=np.int64)])
    wgt = np.concatenate([ew, np.ones(N, np.float32)]).astype(np.float32)
    deg = np.bincount(dst, weights=wgt, minlength=N).astype(np.float32)
    dinv = np.where(deg > 0, 1.0 / np.sqrt(deg), 0.0).astype(np.float32)
    norm = (dinv[src] * wgt * dinv[dst]).astype(np.float32)

    owner = dst // NOWN
    cores = []
    counts = np.zeros((P, NB, NW), np.int64)
    for c in range(P):
        m = owner == c
        es, ed, en = src[m], dst[m] - c * NOWN, norm[m]
        blk, es = _piece_row(es)
        wid = ed // W
        order = np.lexsort((ed, wid, blk))
        es, ed, en, blk, wid = es[order], ed[order], en[order], blk[order], wid[order]
        cnt = np.bincount(blk * NW + wid, minlength=NB * NW).reshape(NB, NW)
        counts[c] = cnt
        cores.append((es, ed, en, blk, wid, cnt))

    # uniform chunk counts across cores (SPMD): K[b, w] 128-edge chunks
    K = -(-counts.max(axis=0) // 128)          # [NB, NW]
    cumKb = np.cumsum(np.concatenate([np.zeros((NB, 1), np.int64), K], axis=1), axis=1)  # excl prefix per block
    totKw = K.sum(axis=0)                      # chunks per window
    GCW = np.concatenate([[0], np.cumsum(totKw)])  # global chunk base per window
    preB = np.cumsum(np.concatenate([np.zeros((1, NW), np.int64), K], axis=0), axis=0)  # [NB+1, NW]
    TOTC = int(K.sum())
    S = [int(K[b].sum()) * 128 for b in range(NB)]

    # bf16 weights, transposed/reshaped for the device
    W1b = np.asarray(W1, np.float32).astype(BF16)            # [512, 256]
    W2b = np.asarray(W2, np.float32).astype(BF16)            # [256, 64]
    w1kc = W1b.reshape(4, 128, H).transpose(1, 0, 2).copy()  # [128, 4, 256]
    w2kc = W2b.reshape(2, 128, C).transpose(1, 0, 2).copy()  # [128, 2, 64]
    b1r = np.asarray(b1, np.float32).astype(BF16).reshape(1, H)
    b2r = np.asarray(b2, np.float32).astype(BF16).reshape(1, C)
    xT = np.ascontiguousarray(x.T).astype(BF16)              # [512, 100000]

    in_maps = []
    for c in range(P):
        es, ed, en, blk, wid, cnt = cores[c]
        im = {"w1kc": w1kc, "w2kc": w2kc, "b1r": b1r, "b2r": b2r,
              "xT_own": np.ascontiguousarray(xT[:, c * NOWN:(c + 1) * NOWN])}
        # position of each real edge inside its (b, w) group
        key = blk * NW + wid
        grp_start = np.concatenate([[0], np.cumsum(cnt.reshape(-1))])[:-1]
        pos_in_grp = np.arange(len(es)) - grp_start[key]
        # destination slot inside the per-block padded stream
        dest = 128 * cumKb[blk, wid] + pos_in_grp
        loc_chunk = dest // 128
        g_slot = GCW[wid] + preB[blk, wid] + (loc_chunk - cumKb[blk, wid])
        dstrel = np.zeros((128, TOTC), BF16)
        nrm = np.zeros((128, TOTC), BF16)
        dstrel[dest % 128, g_slot] = (ed - wid * W).astype(BF16)
        nrm[dest % 128, g_slot] = en.astype(BF16)
        im["dstrel"] = dstrel
        im["nrm"] = nrm
        im["iota128"] = np.broadcast_to(np.arange(128, dtype=np.float32).astype(BF16),
                                        (128, 128)).copy()
        for b in range(NB):
            idx = np.zeros(S[b], np.int16)
            mb = blk == b
            idx[dest[mb]] = es[mb].astype(np.int16)
            wrapped = idx.reshape(S[b] // 16, 16).T            # [16, S/16]
            im[f"idx{b}"] = np.ascontiguousarray(np.tile(wrapped, (8, 1)))
        in_maps.append(im)

    sched = dict(K=K, cumKb=cumKb, GCW=GCW, preB=preB, TOTC=TOTC, S=S)
    return in_maps, sched


def _bcast(ap, n, where):
    """Insert a broadcast (step-0) dim of extent n into a 2D AP at position `where` (1 or 2)."""
    a = ap.ap
    if where == 1:
        new = [a[0], [0, n], a[1]]
    else:
        new = [a[0], a[1], [0, n]]
    return bass.AP(ap.tensor, ap.offset, new)


def _emit_agg(nc, tc, pools, sched, layer, src_dram, b_tile, ones_t, idx_tiles,
              meta, h_dram, o_own, oscr=None):
    """Emit one aggregation layer: gather + Sel matmuls + per-window flush."""
    K, cumKb = sched["K"], sched["cumKb"]
    S, TOTC = sched["S"], sched["TOTC"]
    gpool, selpool, fpool, psum = pools
    pool = fpool
    iota_t, dstrel_t, nrm_t = meta
    ELEMS = H if layer == 1 else 128          # gathered row length (elems)
    RHSW = H if layer == 1 else C             # matmul N dim

    cur_piece = [-1] * NB
    gtiles = [None] * NB
    sel_cur = [-1]
    sel_tile = [None]

    def gather_piece(b, piece):
        n = min(PIECE, S[b] - piece * PIECE)
        t = gpool.tile([128, n // 128, ELEMS], mybir.dt.bfloat16, tag=f"g{b}")
        nc.gpsimd.dma_gather(
            t[:], src_dram[b][:],
            idx_tiles[b][:, (piece * PIECE) // 16:(piece * PIECE + n) // 16],
            n, n, ELEMS, queue_num=b, single_packet=False,
        )
        return t

    def sel_piece(piece):
        # build Sel[k, chunk, m] = nrm * (iota[m] == dstrel) on the vector engine
        n = min(SELP, TOTC - piece * SELP)
        t = selpool.tile([128, n, 128], mybir.dt.bfloat16, tag="sel")
        iota_b = _bcast(iota_t[:], n, 1)
        dr = dstrel_t[:, piece * SELP:piece * SELP + n]
        nr = nrm_t[:, piece * SELP:piece * SELP + n]
        nc.vector.tensor_tensor(t[:], iota_b, _bcast(dr, 128, 2),
                                mybir.AluOpType.is_equal)
        nc.vector.tensor_tensor(t[:], t[:], _bcast(nr, 128, 2),
                                mybir.AluOpType.mult)
        return t

    g = 0
    for w in range(NW):
        acc = psum.tile([128, RHSW], mybir.dt.float32, tag="acc")
        i = 0
        for b in range(NB):
            for k in range(int(K[b, w])):
                bj = int(cumKb[b, w]) + k
                piece, sub = bj // PPC, bj % PPC
                if piece != cur_piece[b]:
                    gtiles[b] = gather_piece(b, piece)
                    cur_piece[b] = piece
                sp, ss = g // SELP, g % SELP
                if sp != sel_cur[0]:
                    sel_tile[0] = sel_piece(sp)
                    sel_cur[0] = sp
                nc.tensor.matmul(acc[:], sel_tile[0][:, ss, :],
                                 gtiles[b][:, sub, 0:RHSW],
                                 start=(i == 0), stop=False)
                i += 1
                g += 1
        # bias via ones-row matmul (K=1)
        nc.tensor.matmul(acc[:], ones_t[0:1, :], b_tile[0:1, :],
                         start=(i == 0), stop=True)

        rows = W if w < NW - 1 else LASTW
        if layer == 1:
            ht = pool.tile([128, H], mybir.dt.bfloat16, tag="hflush")
            nc.vector.tensor_scalar_max(ht[:], acc[:], 0.0)
            nc.sync.dma_start(h_dram[w * W:(w + 1) * W, :], ht[:])
        else:
            # log_softmax: store (o - max) and sum(exp(o - max)); Ln deferred
            o_scr, se_scr = oscr
            mx = pool.tile([128, 1], mybir.dt.float32, tag="lsm")
            nc.vector.tensor_reduce(mx[:], acc[:], mybir.AxisListType.X,
                                    mybir.AluOpType.max)
            nmx = pool.tile([128, 1], mybir.dt.float32, tag="lsn")
            nc.vector.tensor_scalar_mul(nmx[:], mx[:], -1.0)
            et = pool.tile([128, C], mybir.dt.float32, tag="lse")
            nc.scalar.activation(et[:], acc[:],
                                 mybir.ActivationFunctionType.Exp,
                                 bias=nmx[:], accum_out=se_scr[:, w:w + 1])
            nc.vector.tensor_scalar(o_scr[:, w, :], acc[:], mx[:], None,
                                    mybir.AluOpType.subtract)


def _build_program(sched):
    K, S, TOTC = sched["K"], sched["S"], sched["TOTC"]
    nc = bacc.Bacc(None, target_bir_lowering=False, debug=False,
                   num_devices=P, num_swdge_queues=4,
                   dynamic_dma_scratch_size=32768)

    xT_own = nc.dram_tensor("xT_own", [F, NOWN], mybir.dt.bfloat16, kind="ExternalInput")
    w1kc = nc.dram_tensor("w1kc", [128, 4, H], mybir.dt.bfloat16, kind="ExternalInput")
    w2kc = nc.dram_tensor("w2kc", [128, 2, C], mybir.dt.bfloat16, kind="ExternalInput")
    b1r = nc.dram_tensor("b1r", [1, H], mybir.dt.bfloat16, kind="ExternalInput")
    b2r = nc.dram_tensor("b2r", [1, C], mybir.dt.bfloat16, kind="ExternalInput")
    dstrel_d = nc.dram_tensor("dstrel", [128, TOTC], mybir.dt.bfloat16, kind="ExternalInput")
    nrm_d = nc.dram_tensor("nrm", [128, TOTC], mybir.dt.bfloat16, kind="ExternalInput")
    iota_d = nc.dram_tensor("iota128", [128, 128], mybir.dt.bfloat16, kind="ExternalInput")
    idx_dram = [nc.dram_tensor(f"idx{b}", [128, S[b] // 16], mybir.dt.int16, kind="ExternalInput")
                for b in range(NB)]
    o_own = nc.dram_tensor("o_own", [NOWN, C], mybir.dt.float32, kind="ExternalOutput")

    s_own = nc.dram_tensor("s_own", [NOWN, H], mybir.dt.bfloat16)
    s_piece = [nc.dram_tensor(f"s_piece{b}", [BS, H], mybir.dt.bfloat16,
                              addr_space="Shared") for b in range(NB)]
    HPAD = NW * W                                            # 12544
    h_dram = nc.dram_tensor("h_dram", [HPAD, H], mybir.dt.bfloat16)
    t_own = nc.dram_tensor("t_own", [NOWN, 128], mybir.dt.bfloat16)
    t_piece = [nc.dram_tensor(f"t_piece{b}", [BS, 128], mybir.dt.bfloat16,
                              addr_space="Shared") for b in range(NB)]

    with tile.TileContext(nc) as tc:
        with (
            tc.tile_pool(name="const", bufs=1) as constp,
            tc.tile_pool(name="idxp", bufs=1) as idxp,
            tc.tile_pool(name="pool", bufs=2) as pool,
            tc.tile_pool(name="gpool", bufs=2) as gpool,
            tc.tile_pool(name="selpool", bufs=3) as selpool,
            tc.tile_pool(name="fpool", bufs=3) as fpool,
            tc.tile_pool(name="psum", bufs=4, space="PSUM") as psum,
        ):
            w1_t = constp.tile([128, 4, H], mybir.dt.bfloat16)
            nc.sync.dma_start(w1_t[:], w1kc[:])
            w2_t = constp.tile([128, 2, C], mybir.dt.bfloat16)
            nc.sync.dma_start(w2_t[:], w2kc[:])
            b1_t = constp.tile([1, H], mybir.dt.bfloat16)
            nc.sync.dma_start(b1_t[:], b1r[:])
            b2_t = constp.tile([1, C], mybir.dt.bfloat16)
            nc.sync.dma_start(b2_t[:], b2r[:])
            ones_t = constp.tile([1, 128], mybir.dt.bfloat16)
            nc.vector.memset(ones_t[:], 1.0)
            iota_t = constp.tile([128, 128], mybir.dt.bfloat16)
            nc.sync.dma_start(iota_t[:], iota_d[:])
            dstrel_t = constp.tile([128, TOTC], mybir.dt.bfloat16)
            nc.sync.dma_start(dstrel_t[:], dstrel_d[:])
            nrm_t = constp.tile([128, TOTC], mybir.dt.bfloat16)
            nc.sync.dma_start(nrm_t[:], nrm_d[:])
            meta = (iota_t, dstrel_t, nrm_t)
            idx_tiles = []
            for b in range(NB):
                it = idxp.tile([128, S[b] // 16], mybir.dt.int16, tag=f"idx{b}")
                nc.sync.dma_start(it[:], idx_dram[b][:])
                idx_tiles.append(it)

            # ---- phase A: s_own = x_own @ W1 ----
            nt = 0
            while nt * W < NOWN:
                span = min(4, NW - nt)          # node sub-tiles in this big tile
                cols = min(4 * W, NOWN - nt * W)
                xt = pool.tile([128, 4, cols], mybir.dt.bfloat16, tag="xt")
                nc.sync.dma_start(
                    xt[:, :, 0:cols],
                    xT_own[:, nt * W:nt * W + cols].rearrange("(c p) n -> p c n", p=128))
                for s_ in range(span):
                    rows = min(W, cols - s_ * W)
                    acc = psum.tile([128, H], mybir.dt.float32, tag="acc")
                    for kc in range(4):
                        nc.tensor.matmul(acc[0:rows, :],
                                         xt[:, kc, s_ * W:s_ * W + rows],
                                         w1_t[:, kc, :],
                                         start=(kc == 0), stop=(kc == 3))
                    st = pool.tile([128, H], mybir.dt.bfloat16, tag="sflush")
                    nc.vector.tensor_copy(st[0:rows], acc[0:rows])
                    nc.sync.dma_start(s_own[(nt + s_) * W:(nt + s_) * W + rows, :],
                                      st[0:rows, :])
                nt += span

            for b in range(NB):
                nc.gpsimd.collective_compute(
                    "AllGather", mybir.AluOpType.bypass,
                    replica_groups=[list(range(P))],
                    ins=[s_own[b * PQ:(b + 1) * PQ, :]], outs=[s_piece[b][:]])

            # ---- phase B: layer-1 aggregation -> h ----
            _emit_agg(nc, tc, (gpool, selpool, fpool, psum), sched, 1, s_piece,
                      b1_t, ones_t, idx_tiles, meta, h_dram, o_own)

            # ---- phase C: t_own = relu(h) @ W2, AllGather ----
            with tc.tile_pool(name="hT", bufs=1) as hTp:
                hT0 = hTp.tile([128, HPAD], mybir.dt.bfloat16, tag="hT0")
                nc.sync.dma_start(hT0[:], h_dram[:, 0:128], transpose=True)
                hT1 = hTp.tile([128, HPAD], mybir.dt.bfloat16, tag="hT1")
                nc.sync.dma_start(hT1[:], h_dram[:, 128:256], transpose=True)
                for i in range(NW):
                    rows = W if i < NW - 1 else LASTW
                    acc = psum.tile([128, C], mybir.dt.float32, tag="acc")
                    nc.tensor.matmul(acc[0:rows, :], hT0[:, i * W:i * W + rows],
                                     w2_t[:, 0, :], start=True, stop=False)
                    nc.tensor.matmul(acc[0:rows, :], hT1[:, i * W:i * W + rows],
                                     w2_t[:, 1, :], start=False, stop=True)
                    tt = pool.tile([128, 128], mybir.dt.bfloat16, tag="tflush")
                    nc.vector.memset(tt[:, C:128], 0.0)
                    nc.vector.tensor_copy(tt[0:rows, 0:C], acc[0:rows, :])
                    nc.sync.dma_start(t_own[i * W:i * W + rows, :], tt[0:rows, :])

            for b in range(NB):
                nc.gpsimd.collective_compute(
                    "AllGather", mybir.AluOpType.bypass,
                    replica_groups=[list(range(P))],
                    ins=[t_own[b * PQ:(b + 1) * PQ, :]], outs=[t_piece[b][:]])

            # ---- phase D: layer-2 aggregation + log_softmax -> o_own ----
            with tc.tile_pool(name="oscr", bufs=1) as oscrp:
                o_scr = oscrp.tile([128, NW, C], mybir.dt.float32, tag="oscr")
                se_scr = oscrp.tile([128, NW], mybir.dt.float32, tag="sescr")
                _emit_agg(nc, tc, (gpool, selpool, fpool, psum), sched, 2, t_piece,
                          b2_t, ones_t, idx_tiles, meta, h_dram, o_own,
                          oscr=(o_scr, se_scr))
                # deferred log-sum-exp: one Ln over all windows, one subtract
                ln_t = oscrp.tile([128, NW], mybir.dt.float32, tag="lnall")
                nc.scalar.activation(ln_t[:], se_scr[:],
                                     mybir.ActivationFunctionType.Ln)
                res = oscrp.tile([128, NW, C], mybir.dt.float32, tag="res")
                nc.vector.scalar_tensor_tensor(
                    res[:], o_scr[:], 0.0, _bcast(ln_t[:], C, 2),
                    mybir.AluOpType.add, mybir.AluOpType.subtract)
                nc.sync.dma_start(
                    o_own[0:(NW - 1) * W, :].rearrange("(w p) c -> p w c", p=128),
                    res[:, 0:NW - 1, :])
                nc.sync.dma_start(o_own[(NW - 1) * W:NOWN, :],
                                  res[0:LASTW, NW - 1, :])

    nc.compile()
    return nc


def _run(inputs, trace=False, trace_kwargs=None):
    in_maps, sched = _preprocess(**inputs)
    nc = _build_program(sched)
    r = run_bass_kernel_spmd(nc, in_maps, core_ids=list(range(P)),
                             trace=trace, **(trace_kwargs or {}))
    out = np.concatenate([r.results[c]["o_own"] for c in range(P)], axis=0)
    return out, r


def kernel(**inputs) -> np.ndarray:
    out, _ = _run(inputs)
    return out
